# revision 24
# baseline (speedup 1.0000x reference)
"""Multi-headed causal attention on 8 trn2 NeuronCores (Bass/Tile).

Sharding: tensor-parallel over heads — 2 heads per core, all 4 batches.
Everything runs in bf16 on the PE (fp32 PSUM accumulation): host casts
embedded/Wq/Wk/Wv/Wo to bf16, halving HBM traffic and LDWEIGHTS time at
~0.1% relative noise (tolerance is 2e-2).

Structure (j-major attention for collective overlap):
  phase 1: per batch b: Q^T/K^T/V^T projections (heads stacked on the
    partition axis), V transposed to natural layout in bf16, then the
    attention chunks j=0 and j=2 for that batch (their owner-column data
    feeds AllToAll #0).
  a2a #0 fires after b3's j=2 — ~half the attention (all j=1, j=3) plus
    the weight loads for the output projection still overlap it.
  phase 2: attention j=1, j=3 for all batches; then a2a #1, which
    overlaps the q=0 half of the row-sharded output projection.
Attention details:
  - scores in [s_k, s_q] layout, K padded to 128 partition rows by zeros
    so the stacked 2-head Q^T is the moving operand (all matmuls N=512).
  - one exp on ScalarE per score tile; fully-masked leading columns of
    diagonal tiles are skipped in the scores matmul, the exp, AND the AV
    matmul (partial-width PSUM accumulation), so causal masking needs
    only a 128-col affine_select per head on the true diagonal block.
  - V padded to 65 cols (64 V | ones): softmax denominators fall out as
    row 64 of the context accumulator; denominator+context leave PSUM in
    one DVE cast; normalization (fast-reciprocal + partition broadcast +
    multiply) runs off the critical path.
ScalarE does only exp (plus cheap proj-phase copies); DVE takes V/ctx
copies and normalization, keeping the softmax engine-balanced against
the PE inside the attention loop.
"""
import sys

sys.path.insert(0, "/opt/trn_rl_repo")

import numpy as np

import concourse.bass as bass
import concourse.tile as tile
from concourse import bacc, mybir
from concourse.bass_utils import run_bass_kernel_spmd

B, S, D, H, HD = 4, 2048, 1024, 16, 64
NC_ = 8          # cores
PH = 2           # heads per core
SC = 512         # s_q chunk (psum bank width in fp32)
NK = S // 128    # 16 s_k chunks of 128
ND = D // 128    # 8 contraction chunks of 128
F32 = mybir.dt.float32
BF16 = mybir.dt.bfloat16
EXP = mybir.ActivationFunctionType.Exp
GE = mybir.AluOpType.is_ge
PIPE = 2         # scores run this many m-iterations ahead of AV


def build():
    nc = bacc.Bacc("TRN2", target_bir_lowering=False, debug=False, num_devices=NC_)

    emb_t = nc.dram_tensor("embedded_t", [B, D, S], BF16, kind="ExternalInput").ap()
    w_qkv = nc.dram_tensor("w_qkv", [3, ND, 128, 128], BF16, kind="ExternalInput").ap()
    wo_t = nc.dram_tensor("wo_t", [ND, 128, D], BF16, kind="ExternalInput").ap()
    bo_row = nc.dram_tensor("bo_row", [1, D], F32, kind="ExternalInput").ap()
    out_shard = nc.dram_tensor("out_shard", [1024, D], F32, kind="ExternalOutput").ap()

    with tile.TileContext(nc) as tc:
        _build_body(nc, tc, emb_t, w_qkv, wo_t, bo_row, out_shard)

    nc.compile()
    return nc


def _build_body(nc, tc, emb_t, w_qkv, wo_t, bo_row, out_shard):
    from contextlib import ExitStack

    ctx = ExitStack()
    with ctx:
        const = ctx.enter_context(tc.tile_pool(name="const", bufs=1))
        # "mm" slots are sized [128, 1024] f32 (2 PSUM banks): 3x2 + ctx 2x1 = 8
        ps_mm = ctx.enter_context(tc.tile_pool(name="ps_mm", bufs=3, space="PSUM"))
        ps_ctx = ctx.enter_context(tc.tile_pool(name="ps_ctx", bufs=2, space="PSUM"))
        dram = ctx.enter_context(tc.tile_pool(name="dram", bufs=1, space="DRAM"))

        attn_ctx = ExitStack()
        etp = attn_ctx.enter_context(tc.tile_pool(name="etp", bufs=16))
        qtp = attn_ctx.enter_context(tc.tile_pool(name="qtp", bufs=4))
        ktp = attn_ctx.enter_context(tc.tile_pool(name="ktp", bufs=8))
        vtp = attn_ctx.enter_context(tc.tile_pool(name="vtp", bufs=1))
        vsb = attn_ctx.enter_context(tc.tile_pool(name="vsb", bufs=8))
        exp_p = attn_ctx.enter_context(tc.tile_pool(name="exp_p", bufs=4))
        cu_p = attn_ctx.enter_context(tc.tile_pool(name="cu_p", bufs=2))
        cn_p = attn_ctx.enter_context(tc.tile_pool(name="cn_p", bufs=6))
        rc_p = attn_ctx.enter_context(tc.tile_pool(name="rc_p", bufs=1))
        rb_p = attn_ctx.enter_context(tc.tile_pool(name="rb_p", bufs=1))

        # ---- weights first (tiny), then batch-0 activation prefetch ----
        wq_all = const.tile([128, 24, 128], BF16, tag="wq_all")
        for p in range(3):
            nc.sync.dma_start(out=wq_all[:, 8 * p:8 * (p + 1), :],
                              in_=bass.AP(
                tensor=w_qkv.tensor, offset=131072 * p,
                ap=[[128, 128], [16384, 8], [1, 128]]))
        wq_sb = [[wq_all[:, 8 * p + c, :] for c in range(ND)] for p in range(3)]

        et0 = {}
        for j4 in range(4):
            for c in range(ND):
                t = etp.tile([128, SC], BF16, tag="et", name=f"et0_{j4}_{c}")
                eng = nc.sync if (c % 2 == 0) else nc.scalar
                eng.dma_start(
                    out=t[:], in_=emb_t[0, 128 * c:128 * (c + 1),
                                        SC * j4:SC * (j4 + 1)])
                et0[(j4, c)] = t

        bo_sb = const.tile([1, D], F32, tag="bo1")
        nc.sync.dma_start(out=bo_sb[:], in_=bo_row[:])
        bo_b = const.tile([128, D], F32, tag="bob")
        nc.gpsimd.partition_broadcast(bo_b[:], bo_sb[:])

        ones_f32 = const.tile([128, 1], F32, tag="ones_f32")
        nc.vector.memset(ones_f32[:], 1.0)
        ones_r = const.tile([128, 1], BF16, tag="ones_r")
        nc.vector.tensor_copy(ones_r[:], ones_f32[:])

        o64_f = const.tile([1, 64], F32, tag="o64_f")
        nc.vector.memset(o64_f[:], 1.0)
        ones1_64 = const.tile([1, 64], BF16, tag="ones1_64")
        nc.vector.tensor_copy(ones1_64[:], o64_f[:])

        # ident builds through an all-ones -> lower-triangle intermediate;
        # the lower-tri state doubles as the bf16 causal mask `tri`
        # (tri[k, q] = 1 iff k <= q) for DVE-side masking in j=3 units
        ident = const.tile([128, 128], F32, tag="ident")
        nc.gpsimd.memset(ident[:], 1.0)
        nc.gpsimd.affine_select(out=ident[:], in_=ident[:], compare_op=GE,
                                fill=0.0, base=0, pattern=[[1, 128]],
                                channel_multiplier=-1)
        tri = const.tile([128, 128], BF16, tag="tri")
        nc.vector.tensor_copy(tri[:], ident[:])
        nc.gpsimd.affine_select(out=ident[:], in_=ident[:], compare_op=GE,
                                fill=0.0, base=0, pattern=[[-1, 128]],
                                channel_multiplier=1)

        # output-projection weights: loaded mid-attention (after a2a #0)
        wot_sb = [const.tile([128, D], BF16, tag=f"wo{c}", name=f"wo{c}")
                  for c in range(ND)]

        a2a_in = [dram.tile([NC_, 128, 512], BF16, tag=f"a2a_in{q}",
                            name=f"a2a_in{q}") for q in range(2)]
        a2a_out = [dram.tile([NC_, 128, 512], BF16, tag=f"a2a_out{q}",
                             name=f"a2a_out{q}") for q in range(2)]

        def emit_a2a(q):
            nc.gpsimd.collective_compute(
                "AllToAll", mybir.AluOpType.bypass,
                replica_groups=[list(range(NC_))],
                ins=[a2a_in[q].opt()], outs=[a2a_out[q].opt()])

        qt, kts, v01 = {}, {}, {}

        def proj(b):
            qt_b = qtp.tile([128, S], BF16, tag="qt", name=f"qt{b}")
            # per-head K^T padded to K=128 with zero rows for the other head
            kt0 = ktp.tile([128, S], BF16, tag="kt", name=f"kt0_{b}")
            kt1 = ktp.tile([128, S], BF16, tag="kt", name=f"kt1_{b}")
            nc.vector.memset(kt0[64:128, :], 0.0)
            nc.vector.memset(kt1[0:64, :], 0.0)
            vt = vtp.tile([128, S], F32, tag="vt", name=f"vt{b}")
            for j4 in range(4):          # s chunks of 512
                sl = slice(SC * j4, SC * (j4 + 1))
                for p in range(3):
                    ps = ps_mm.tile([128, SC], F32, tag="mm",
                                    name=f"pj{b}_{j4}_{p}")
                    for c in range(ND):
                        rhs = (et0[(j4, c)][:] if b == 0
                               else et[c][:, sl])
                        nc.tensor.matmul(
                            ps[:], lhsT=wq_sb[p][c], rhs=rhs,
                            start=(c == 0), stop=(c == ND - 1))
                    if p == 0:
                        nc.scalar.copy(qt_b[:, sl], ps[:])
                    elif p == 1:
                        nc.scalar.copy(kt0[0:64, sl], ps[0:64, :])
                        nc.scalar.copy(kt1[64:128, sl], ps[64:128, :])
                    else:
                        nc.vector.tensor_copy(vt[:, sl], ps[:])
            # V natural layout: 64 V cols | ones col (for softmax denom).
            # 128-col row stride keeps the AV weight loads 256B-aligned;
            # cols 65:128 are never read (the AV lhsT reads [0:65)).
            v_b = [vsb.tile([128, NK, 128], BF16, tag="v01", name=f"v{b}_{h}")
                   for h in range(PH)]
            for sk in range(NK):
                pt = ps_mm.tile([128, 128], F32, tag="mm", name=f"tr{b}_{sk}")
                nc.tensor.transpose(pt[:], vt[:, 128 * sk:128 * (sk + 1)],
                                    ident[:])
                for h in range(PH):
                    nc.vector.tensor_copy(v_b[h][:, sk, 0:64],
                                          pt[:, 64 * h:64 * (h + 1)])
                    nc.vector.tensor_copy(v_b[h][:, sk, 64:65], ones_r[:])
            qt[b], kts[b], v01[b] = qt_b, [kt0, kt1], v_b

        def attn(b, j):
            pool_free = (j == 3)   # no Pool ops once a2a #0 holds the engine
            mtop = 4 * j + 4
            qt_b, kts_b, v_b = qt[b], kts[b], v01[b]
            ctx_ps = [ps_ctx.tile([65, SC], F32, tag="ctx",
                                  name=f"ctx{b}_{j}_{h}")
                      for h in range(PH)]
            exq = []   # (m, c0, ex) awaiting AV

            def emit_scores(m):
                # cols [0, c0) of each half are fully causal-masked
                c0 = max(0, 128 * m - SC * j)
                psc = ps_mm.tile([128, 2 * SC], F32, tag="mm",
                                 name=f"sc{b}_{j}_{m}")
                for h in range(PH):
                    nc.tensor.matmul(
                        psc[:, SC * h + c0:SC * (h + 1)],
                        lhsT=kts_b[h][:, 128 * m:128 * (m + 1)],
                        rhs=qt_b[:, SC * j + c0:SC * (j + 1)],
                        start=True, stop=True)
                ex = exp_p.tile([128, 2 * SC], BF16, tag="ex",
                                name=f"ex{b}_{j}_{m}")
                if m >= 4 * j:
                    # diagonal tile: exp skips the masked lead, then a
                    # 128-col triangle select per head (the lead columns
                    # are never read: the AV matmul skips them too)
                    if c0 == 0:
                        nc.scalar.activation(out=ex[:], in_=psc[:],
                                             func=EXP, scale=0.125)
                    else:
                        for h in range(PH):
                            nc.scalar.activation(
                                out=ex[:, SC * h + c0:SC * (h + 1)],
                                in_=psc[:, SC * h + c0:SC * (h + 1)],
                                func=EXP, scale=0.125)
                    for h in range(PH):
                        if pool_free:
                            nc.vector.tensor_mul(
                                ex[:, SC * h + c0:SC * h + c0 + 128],
                                ex[:, SC * h + c0:SC * h + c0 + 128],
                                tri[:])
                        else:
                            nc.gpsimd.affine_select(
                                out=ex[:, SC * h + c0:SC * h + c0 + 128],
                                in_=ex[:, SC * h + c0:SC * h + c0 + 128],
                                compare_op=GE, fill=0.0, base=0,
                                pattern=[[1, 128]], channel_multiplier=-1)
                else:
                    nc.scalar.activation(out=ex[:], in_=psc[:],
                                         func=EXP, scale=0.125)
                exq.append((m, c0, ex))

            def emit_av():
                m_av, c0, ex = exq.pop(0)
                for h in range(PH):
                    nc.tensor.matmul(
                        ctx_ps[h][:, c0:], lhsT=v_b[h][:, m_av, 0:65],
                        rhs=ex[:, SC * h + c0:SC * (h + 1)],
                        start=(m_av == 0), stop=(m_av == mtop - 1))

            for m in range(mtop):
                emit_scores(m)
                if len(exq) > PIPE:
                    emit_av()
            while exq:
                emit_av()

            # drain PSUM fast, normalize off the critical path
            # owner-block col half: even j -> a2a buffer 0, odd j -> 1
            o, q0 = 2 * b + j // 2, j % 2

            def normalize():
                for h in range(PH):
                    cudn = cu_p.tile([65, SC], F32, tag="cu",
                                     name=f"cu{b}_{j}_{h}")
                    nc.vector.tensor_copy(cudn[:], ctx_ps[h][:])
                    # denominator row must reach partition 0 before the DVE
                    # reciprocal: its custom op breaks on partition-shifted
                    # inputs on hw (ACT copies shift partitions fine)
                    dn = rc_p.tile([1, SC], F32, tag="dn")
                    nc.scalar.copy(dn[:], ctx_ps[h][64:65, :])
                    rc = dn
                    nc.vector.reciprocal_approx_fast(rc[:], dn[:])
                    cn = cn_p.tile([64, SC], BF16, tag="cn")
                    if pool_free:
                        # broadcast 1/denom via a rank-1 PE matmul (Pool is
                        # occupied by the collective during j=3)
                        rcb = rb_p.tile([1, SC], BF16, tag="rcb")
                        nc.vector.tensor_copy(rcb[:], rc[:])
                        rbp = ps_mm.tile([64, SC], F32, tag="mm",
                                         name=f"rb{b}_{j}_{h}")
                        nc.tensor.matmul(rbp[:], lhsT=ones1_64[:],
                                         rhs=rcb[:], start=True, stop=True)
                        nc.vector.tensor_mul(cn[:], cudn[0:64, :], rbp[:])
                    else:
                        rb = rb_p.tile([64, SC], F32, tag="rb")
                        nc.gpsimd.partition_broadcast(rb[:], rc[:])
                        nc.vector.tensor_mul(cn[:], cudn[0:64, :], rb[:])
                    nc.sync.dma_start(
                        out=a2a_in[q0][o, 64 * h:64 * (h + 1), :], in_=cn[:])
            if pool_free:
                return normalize
            normalize()

        # ---- phase 1: projections + attention j=0, j=2 per batch ----
        for b in range(B):
            proj(b)
            if b + 1 < B:
                et = {}
                for c in range(ND):
                    t = etp.tile([128, S], BF16, tag="et", name=f"et{b+1}_{c}")
                    nc.sync.dma_start(
                        out=t[:], in_=emb_t[b + 1, 128 * c:128 * (c + 1), :])
                    et[c] = t
            attn(b, 0)
            attn(b, 2)
        # output-projection weights stream in while j=1/j=3 run
        for c in range(ND):
            nc.sync.dma_start(out=wot_sb[c][:], in_=wo_t[c])

        # ---- phase 2: attention j=1, j=3 ----
        for b in range(B):
            attn(b, 1)
        # a2a #0 triggers here: its wait (all j=0/j=2 writes) is satisfied
        # long before the Pool queue reaches it, so the in-order Pool queue
        # never blocks the j=3 selects behind it
        emit_a2a(0)
        fin = None
        for b in range(B):
            nxt = attn(b, 3)
            if fin is not None:
                fin()
            fin = nxt
        fin()

        # ---- remaining all-to-all + row-sharded output projection ----
        attn_ctx.close()

        cat_p = ctx.enter_context(tc.tile_pool(name="cat_p", bufs=16))
        ob_p = ctx.enter_context(tc.tile_pool(name="ob_p", bufs=3))
        all_cats = {}
        # q=0 loads issue from the gpsimd queue ahead of collective #1 so
        # they hit the DMA hardware before the a2a occupies it
        for q in range(2):
            all_cats[q] = []
            for r in range(NC_):
                ct = cat_p.tile([128, 512], BF16, tag=f"cat{q}",
                                name=f"cat{q}_{r}")
                eng = nc.gpsimd if q == 0 else nc.sync
                eng.dma_start(out=ct[:], in_=a2a_out[q][r])
                all_cats[q].append(ct)
            if q == 0:
                emit_a2a(1)
        for q in range(2):
            cats = all_cats[q]
            for sq in (4 * q, 4 * q + 1, 4 * q + 2, 4 * q + 3):
                lo = 128 * (sq % 4)
                for n in range(2):
                    po = ps_mm.tile([128, SC], F32, tag="mm",
                                    name=f"po{sq}_{n}")
                    for kp in range(ND):
                        nc.tensor.matmul(
                            po[:], lhsT=cats[kp][:, lo:lo + 128],
                            rhs=wot_sb[kp][:, SC * n:SC * (n + 1)],
                            start=(kp == 0), stop=(kp == ND - 1))
                    ob = ob_p.tile([128, SC], F32, tag="ob")
                    nc.vector.tensor_add(ob[:], po[:],
                                         bo_b[:, SC * n:SC * (n + 1)])
                    nc.sync.dma_start(
                        out=out_shard[128 * sq:128 * (sq + 1),
                                      SC * n:SC * (n + 1)],
                        in_=ob[:])


_NC_CACHE = None


def _get_nc():
    global _NC_CACHE
    if _NC_CACHE is None:
        _NC_CACHE = build()
    return _NC_CACHE


def kernel(embedded, Wq, Wk, Wv, Wo, bo, _trace=False):
    import ml_dtypes
    bf16 = ml_dtypes.bfloat16
    embedded = np.ascontiguousarray(np.asarray(embedded, np.float32))
    emb_t = np.ascontiguousarray(embedded.transpose(0, 2, 1)).astype(bf16)
    W = np.stack([np.asarray(Wq), np.asarray(Wk), np.asarray(Wv)]).astype(np.float32)
    wo_t = np.ascontiguousarray(np.asarray(Wo, np.float32).T).astype(
        bf16).reshape(ND, 128, D)
    bo_row = np.asarray(bo, np.float32).reshape(1, D)

    in_maps = []
    for c in range(NC_):
        w = W[:, 2 * c:2 * c + 2]                  # [3, 2, D, HD]
        w = np.ascontiguousarray(w.transpose(0, 2, 1, 3)).reshape(
            3, ND, 128, 128).astype(bf16)
        in_maps.append({
            "embedded_t": emb_t,
            "w_qkv": w,
            "wo_t": wo_t,
            "bo_row": bo_row,
        })

    nc = _get_nc()
    res = run_bass_kernel_spmd(nc, in_maps, core_ids=list(range(NC_)),
                               trace=_trace)

    out = np.empty((B, S, D), np.float32)
    for c in range(NC_):
        s0 = (c % 2) * 1024
        out[c // 2, s0:s0 + 1024, :] = res.results[c]["out_shard"]
    if _trace:
        return out, res
    return out


# revision 26
# speedup vs baseline: 1.0511x; 1.0511x over previous
"""Multi-headed causal attention on 8 trn2 NeuronCores (Bass/Tile).

Sharding: tensor-parallel over heads — 2 heads per core, all 4 batches.
Everything runs in bf16 on the PE (fp32 PSUM accumulation): host casts
embedded/Wq/Wk/Wv/Wo to bf16, halving HBM traffic and LDWEIGHTS time at
~0.1% relative noise (tolerance is 2e-2).

Structure (j-major attention for collective overlap):
  phase 1: per batch b: Q^T/K^T/V^T projections (heads stacked on the
    partition axis), V transposed to natural layout in bf16, then the
    attention chunks j=0 and j=2 for that batch (their owner-column data
    feeds AllToAll #0).
  a2a #0 fires after b3's j=2 — ~half the attention (all j=1, j=3) plus
    the weight loads for the output projection still overlap it.
  phase 2: attention j=1, j=3 for all batches; then a2a #1, which
    overlaps the q=0 half of the row-sharded output projection.
Attention details:
  - scores in [s_k, s_q] layout, K padded to 128 partition rows by zeros
    so the stacked 2-head Q^T is the moving operand (all matmuls N=512).
  - one exp on ScalarE per score tile; fully-masked leading columns of
    diagonal tiles are skipped in the scores matmul, the exp, AND the AV
    matmul (partial-width PSUM accumulation), so causal masking needs
    only a 128-col affine_select per head on the true diagonal block.
  - V padded to 65 cols (64 V | ones): softmax denominators fall out as
    row 64 of the context accumulator; denominator+context leave PSUM in
    one DVE cast; normalization (fast-reciprocal + partition broadcast +
    multiply) runs off the critical path.
ScalarE does only exp (plus cheap proj-phase copies); DVE takes V/ctx
copies and normalization, keeping the softmax engine-balanced against
the PE inside the attention loop.
"""
import sys

sys.path.insert(0, "/opt/trn_rl_repo")

import numpy as np

import concourse.bass as bass
import concourse.tile as tile
from concourse import bacc, mybir
from concourse.bass_utils import run_bass_kernel_spmd

B, S, D, H, HD = 4, 2048, 1024, 16, 64
NC_ = 8          # cores
PH = 2           # heads per core
SC = 512         # s_q chunk (psum bank width in fp32)
NK = S // 128    # 16 s_k chunks of 128
ND = D // 128    # 8 contraction chunks of 128
F32 = mybir.dt.float32
BF16 = mybir.dt.bfloat16
EXP = mybir.ActivationFunctionType.Exp
GE = mybir.AluOpType.is_ge
PIPE = 2         # scores run this many m-iterations ahead of AV


def build():
    nc = bacc.Bacc("TRN2", target_bir_lowering=False, debug=False, num_devices=NC_)

    emb_t = nc.dram_tensor("embedded_t", [B, D, S], BF16, kind="ExternalInput").ap()
    w_qkv = nc.dram_tensor("w_qkv", [3, ND, 128, 128], BF16, kind="ExternalInput").ap()
    wo_t = nc.dram_tensor("wo_t", [ND, 128, D], BF16, kind="ExternalInput").ap()
    bo_row = nc.dram_tensor("bo_row", [1, D], F32, kind="ExternalInput").ap()
    out_shard = nc.dram_tensor("out_shard", [1024, D], F32, kind="ExternalOutput").ap()

    with tile.TileContext(nc) as tc:
        _build_body(nc, tc, emb_t, w_qkv, wo_t, bo_row, out_shard)

    nc.compile()
    return nc


def _build_body(nc, tc, emb_t, w_qkv, wo_t, bo_row, out_shard):
    from contextlib import ExitStack

    ctx = ExitStack()
    with ctx:
        const = ctx.enter_context(tc.tile_pool(name="const", bufs=1))
        # "mm" slots are sized [128, 1024] f32 (2 PSUM banks): 3x2 + ctx 2x1 = 8
        ps_mm = ctx.enter_context(tc.tile_pool(name="ps_mm", bufs=3, space="PSUM"))
        ps_ctx = ctx.enter_context(tc.tile_pool(name="ps_ctx", bufs=2, space="PSUM"))
        dram = ctx.enter_context(tc.tile_pool(name="dram", bufs=1, space="DRAM"))

        attn_ctx = ExitStack()
        etp = attn_ctx.enter_context(tc.tile_pool(name="etp", bufs=15))
        qtp = attn_ctx.enter_context(tc.tile_pool(name="qtp", bufs=4))
        ktp = attn_ctx.enter_context(tc.tile_pool(name="ktp", bufs=8))
        vtp = attn_ctx.enter_context(tc.tile_pool(name="vtp", bufs=1))
        vsb = attn_ctx.enter_context(tc.tile_pool(name="vsb", bufs=8))
        exp_p = attn_ctx.enter_context(tc.tile_pool(name="exp_p", bufs=4))
        cu_p = attn_ctx.enter_context(tc.tile_pool(name="cu_p", bufs=2))
        cn_p = attn_ctx.enter_context(tc.tile_pool(name="cn_p", bufs=10))
        rc_p = attn_ctx.enter_context(tc.tile_pool(name="rc_p", bufs=1))
        rb_p = attn_ctx.enter_context(tc.tile_pool(name="rb_p", bufs=1))

        # ---- weights first (tiny), then batch-0 activation prefetch ----
        wq_all = const.tile([128, 24, 128], BF16, tag="wq_all")
        for p in range(3):
            nc.sync.dma_start(out=wq_all[:, 8 * p:8 * (p + 1), :],
                              in_=bass.AP(
                tensor=w_qkv.tensor, offset=131072 * p,
                ap=[[128, 128], [16384, 8], [1, 128]]))
        wq_sb = [[wq_all[:, 8 * p + c, :] for c in range(ND)] for p in range(3)]

        et0 = {}
        for j4 in range(4):
            for c in range(ND):
                t = etp.tile([128, SC], BF16, tag="et", name=f"et0_{j4}_{c}")
                eng = nc.sync if (c % 2 == 0) else nc.scalar
                eng.dma_start(
                    out=t[:], in_=emb_t[0, 128 * c:128 * (c + 1),
                                        SC * j4:SC * (j4 + 1)])
                et0[(j4, c)] = t

        bo_sb = const.tile([1, D], F32, tag="bo1")
        nc.sync.dma_start(out=bo_sb[:], in_=bo_row[:])
        bo_b = const.tile([128, D], F32, tag="bob")
        nc.gpsimd.partition_broadcast(bo_b[:], bo_sb[:])

        ones_f32 = const.tile([128, 1], F32, tag="ones_f32")
        nc.vector.memset(ones_f32[:], 1.0)
        ones_r = const.tile([128, 1], BF16, tag="ones_r")
        nc.vector.tensor_copy(ones_r[:], ones_f32[:])

        o64_f = const.tile([1, 64], F32, tag="o64_f")
        nc.vector.memset(o64_f[:], 1.0)
        ones1_64 = const.tile([1, 64], BF16, tag="ones1_64")
        nc.vector.tensor_copy(ones1_64[:], o64_f[:])

        # ident builds through an all-ones -> lower-triangle intermediate;
        # the lower-tri state doubles as the bf16 causal mask `tri`
        # (tri[k, q] = 1 iff k <= q) for DVE-side masking in j=3 units
        ident = const.tile([128, 128], F32, tag="ident")
        nc.gpsimd.memset(ident[:], 1.0)
        nc.gpsimd.affine_select(out=ident[:], in_=ident[:], compare_op=GE,
                                fill=0.0, base=0, pattern=[[1, 128]],
                                channel_multiplier=-1)
        tri = const.tile([128, 128], BF16, tag="tri")
        nc.vector.tensor_copy(tri[:], ident[:])
        nc.gpsimd.affine_select(out=ident[:], in_=ident[:], compare_op=GE,
                                fill=0.0, base=0, pattern=[[-1, 128]],
                                channel_multiplier=1)

        # output-projection weights: loaded mid-attention (after a2a #0)
        wot_sb = [const.tile([128, D], BF16, tag=f"wo{c}", name=f"wo{c}")
                  for c in range(ND)]

        a2a_in = [dram.tile([NC_, 128, 512], BF16, tag=f"a2a_in{q}",
                            name=f"a2a_in{q}") for q in range(2)]
        a2a_out = [dram.tile([NC_, 128, 512], BF16, tag=f"a2a_out{q}",
                             name=f"a2a_out{q}") for q in range(2)]

        def emit_a2a(q):
            nc.gpsimd.collective_compute(
                "AllToAll", mybir.AluOpType.bypass,
                replica_groups=[list(range(NC_))],
                ins=[a2a_in[q].opt()], outs=[a2a_out[q].opt()])

        qt, kts, v01 = {}, {}, {}

        def proj(b):
            qt_b = qtp.tile([128, S], BF16, tag="qt", name=f"qt{b}")
            # per-head K^T padded to K=128 with zero rows for the other head
            kt0 = ktp.tile([128, S], BF16, tag="kt", name=f"kt0_{b}")
            kt1 = ktp.tile([128, S], BF16, tag="kt", name=f"kt1_{b}")
            nc.vector.memset(kt0[64:128, :], 0.0)
            nc.vector.memset(kt1[0:64, :], 0.0)
            vt = vtp.tile([128, S], F32, tag="vt", name=f"vt{b}")
            for j4 in range(4):          # s chunks of 512
                sl = slice(SC * j4, SC * (j4 + 1))
                for p in range(3):
                    ps = ps_mm.tile([128, SC], F32, tag="mm",
                                    name=f"pj{b}_{j4}_{p}")
                    for c in range(ND):
                        rhs = (et0[(j4, c)][:] if b == 0
                               else et[c][:, sl])
                        nc.tensor.matmul(
                            ps[:], lhsT=wq_sb[p][c], rhs=rhs,
                            start=(c == 0), stop=(c == ND - 1))
                    if p == 0:
                        nc.scalar.copy(qt_b[:, sl], ps[:])
                    elif p == 1:
                        nc.scalar.copy(kt0[0:64, sl], ps[0:64, :])
                        nc.scalar.copy(kt1[64:128, sl], ps[64:128, :])
                    else:
                        nc.vector.tensor_copy(vt[:, sl], ps[:])
            # V natural layout: 64 V cols | ones col (for softmax denom).
            # 128-col row stride keeps the AV weight loads 256B-aligned;
            # cols 65:128 are never read (the AV lhsT reads [0:65)).
            v_b = [vsb.tile([128, NK, 128], BF16, tag="v01", name=f"v{b}_{h}")
                   for h in range(PH)]
            for sk in range(NK):
                pt = ps_mm.tile([128, 128], F32, tag="mm", name=f"tr{b}_{sk}")
                nc.tensor.transpose(pt[:], vt[:, 128 * sk:128 * (sk + 1)],
                                    ident[:])
                for h in range(PH):
                    nc.vector.tensor_copy(v_b[h][:, sk, 0:64],
                                          pt[:, 64 * h:64 * (h + 1)])
                    nc.vector.tensor_copy(v_b[h][:, sk, 64:65], ones_r[:])
            qt[b], kts[b], v01[b] = qt_b, [kt0, kt1], v_b

        def attn(b, j):
            pool_free = (j == 3)   # no Pool ops once a2a #0 holds the engine
            mtop = 4 * j + 4
            qt_b, kts_b, v_b = qt[b], kts[b], v01[b]
            ctx_ps = [ps_ctx.tile([65, SC], F32, tag="ctx",
                                  name=f"ctx{b}_{j}_{h}")
                      for h in range(PH)]
            exq = []   # (m, c0, ex) awaiting AV

            def emit_scores(m):
                # cols [0, c0) of each half are fully causal-masked
                c0 = max(0, 128 * m - SC * j)
                psc = ps_mm.tile([128, 2 * SC], F32, tag="mm",
                                 name=f"sc{b}_{j}_{m}")
                for h in range(PH):
                    nc.tensor.matmul(
                        psc[:, SC * h + c0:SC * (h + 1)],
                        lhsT=kts_b[h][:, 128 * m:128 * (m + 1)],
                        rhs=qt_b[:, SC * j + c0:SC * (j + 1)],
                        start=True, stop=True)
                ex = exp_p.tile([128, 2 * SC], BF16, tag="ex",
                                name=f"ex{b}_{j}_{m}")
                if m >= 4 * j:
                    # diagonal tile: exp skips the masked lead, then a
                    # 128-col triangle select per head (the lead columns
                    # are never read: the AV matmul skips them too)
                    if c0 == 0:
                        nc.scalar.activation(out=ex[:], in_=psc[:],
                                             func=EXP, scale=0.125)
                    else:
                        for h in range(PH):
                            nc.scalar.activation(
                                out=ex[:, SC * h + c0:SC * (h + 1)],
                                in_=psc[:, SC * h + c0:SC * (h + 1)],
                                func=EXP, scale=0.125)
                    for h in range(PH):
                        if pool_free:
                            nc.vector.tensor_mul(
                                ex[:, SC * h + c0:SC * h + c0 + 128],
                                ex[:, SC * h + c0:SC * h + c0 + 128],
                                tri[:])
                        else:
                            nc.gpsimd.affine_select(
                                out=ex[:, SC * h + c0:SC * h + c0 + 128],
                                in_=ex[:, SC * h + c0:SC * h + c0 + 128],
                                compare_op=GE, fill=0.0, base=0,
                                pattern=[[1, 128]], channel_multiplier=-1)
                else:
                    nc.scalar.activation(out=ex[:], in_=psc[:],
                                         func=EXP, scale=0.125)
                exq.append((m, c0, ex))

            def emit_av():
                m_av, c0, ex = exq.pop(0)
                for h in range(PH):
                    nc.tensor.matmul(
                        ctx_ps[h][:, c0:], lhsT=v_b[h][:, m_av, 0:65],
                        rhs=ex[:, SC * h + c0:SC * (h + 1)],
                        start=(m_av == 0), stop=(m_av == mtop - 1))

            for m in range(mtop):
                emit_scores(m)
                if len(exq) > PIPE:
                    emit_av()
            while exq:
                emit_av()

            # drain PSUM fast, normalize off the critical path
            # owner-block col half: even j -> a2a buffer 0, odd j -> 1
            o, q0 = 2 * b + j // 2, j % 2

            def normalize():
                for h in range(PH):
                    cudn = cu_p.tile([65, SC], F32, tag="cu",
                                     name=f"cu{b}_{j}_{h}")
                    nc.vector.tensor_copy(cudn[:], ctx_ps[h][:])
                    # denominator row must reach partition 0 before the DVE
                    # reciprocal: its custom op breaks on partition-shifted
                    # inputs on hw (ACT copies shift partitions fine)
                    dn = rc_p.tile([1, SC], F32, tag="dn")
                    nc.scalar.copy(dn[:], ctx_ps[h][64:65, :])
                    rc = dn
                    nc.vector.reciprocal_approx_fast(rc[:], dn[:])
                    cn = cn_p.tile([64, SC], BF16, tag="cn")
                    if pool_free:
                        # broadcast 1/denom via a rank-1 PE matmul (Pool is
                        # occupied by the collective during j=3)
                        rcb = rb_p.tile([1, SC], BF16, tag="rcb")
                        nc.vector.tensor_copy(rcb[:], rc[:])
                        rbp = ps_mm.tile([64, SC], F32, tag="mm",
                                         name=f"rb{b}_{j}_{h}")
                        nc.tensor.matmul(rbp[:], lhsT=ones1_64[:],
                                         rhs=rcb[:], start=True, stop=True)
                        nc.vector.tensor_mul(cn[:], cudn[0:64, :], rbp[:])
                    else:
                        rb = rb_p.tile([64, SC], F32, tag="rb")
                        nc.gpsimd.partition_broadcast(rb[:], rc[:])
                        nc.vector.tensor_mul(cn[:], cudn[0:64, :], rb[:])
                    nc.sync.dma_start(
                        out=a2a_in[q0][o, 64 * h:64 * (h + 1), :], in_=cn[:])
            if pool_free:
                return normalize
            normalize()

        # ---- phase 1: projections + attention j=0, j=2 per batch ----
        for b in range(B):
            proj(b)
            if b + 1 < B:
                et = {}
                for c in range(ND):
                    t = etp.tile([128, S], BF16, tag="et", name=f"et{b+1}_{c}")
                    nc.sync.dma_start(
                        out=t[:], in_=emb_t[b + 1, 128 * c:128 * (c + 1), :])
                    et[c] = t
            attn(b, 0)
            attn(b, 2)
        # output-projection weights stream in while j=1/j=3 run
        for c in range(ND):
            nc.sync.dma_start(out=wot_sb[c][:], in_=wo_t[c])

        # ---- phase 2: attention j=1, j=3 ----
        for b in range(B):
            attn(b, 1)
        # a2a #0 triggers here: its wait (all j=0/j=2 writes) is satisfied
        # long before the Pool queue reaches it, so the in-order Pool queue
        # never blocks the j=3 selects behind it
        emit_a2a(0)
        fin = None
        for b in range(B):
            nxt = attn(b, 3)
            if fin is not None:
                fin()
            fin = nxt
        fin()

        # ---- remaining all-to-all + row-sharded output projection ----
        attn_ctx.close()

        cat_p = ctx.enter_context(tc.tile_pool(name="cat_p", bufs=16))
        ob_p = ctx.enter_context(tc.tile_pool(name="ob_p", bufs=3))
        all_cats = {}
        # q=0 loads issue from the gpsimd queue ahead of collective #1 so
        # they hit the DMA hardware before the a2a occupies it
        for q in range(2):
            all_cats[q] = []
            for r in range(NC_):
                ct = cat_p.tile([128, 512], BF16, tag=f"cat{q}",
                                name=f"cat{q}_{r}")
                eng = nc.gpsimd if q == 0 else nc.sync
                eng.dma_start(out=ct[:], in_=a2a_out[q][r])
                all_cats[q].append(ct)
            if q == 0:
                emit_a2a(1)
        for q in range(2):
            cats = all_cats[q]
            for sq in (4 * q, 4 * q + 1, 4 * q + 2, 4 * q + 3):
                lo = 128 * (sq % 4)
                for n in range(2):
                    po = ps_mm.tile([128, SC], F32, tag="mm",
                                    name=f"po{sq}_{n}")
                    for kp in range(ND):
                        nc.tensor.matmul(
                            po[:], lhsT=cats[kp][:, lo:lo + 128],
                            rhs=wot_sb[kp][:, SC * n:SC * (n + 1)],
                            start=(kp == 0), stop=(kp == ND - 1))
                    ob = ob_p.tile([128, SC], F32, tag="ob")
                    nc.vector.tensor_add(ob[:], po[:],
                                         bo_b[:, SC * n:SC * (n + 1)])
                    nc.sync.dma_start(
                        out=out_shard[128 * sq:128 * (sq + 1),
                                      SC * n:SC * (n + 1)],
                        in_=ob[:])


_NC_CACHE = None


def _get_nc():
    global _NC_CACHE
    if _NC_CACHE is None:
        _NC_CACHE = build()
    return _NC_CACHE


def kernel(embedded, Wq, Wk, Wv, Wo, bo, _trace=False):
    import ml_dtypes
    bf16 = ml_dtypes.bfloat16
    embedded = np.ascontiguousarray(np.asarray(embedded, np.float32))
    emb_t = np.ascontiguousarray(embedded.transpose(0, 2, 1)).astype(bf16)
    W = np.stack([np.asarray(Wq), np.asarray(Wk), np.asarray(Wv)]).astype(np.float32)
    wo_t = np.ascontiguousarray(np.asarray(Wo, np.float32).T).astype(
        bf16).reshape(ND, 128, D)
    bo_row = np.asarray(bo, np.float32).reshape(1, D)

    in_maps = []
    for c in range(NC_):
        w = W[:, 2 * c:2 * c + 2]                  # [3, 2, D, HD]
        w = np.ascontiguousarray(w.transpose(0, 2, 1, 3)).reshape(
            3, ND, 128, 128).astype(bf16)
        in_maps.append({
            "embedded_t": emb_t,
            "w_qkv": w,
            "wo_t": wo_t,
            "bo_row": bo_row,
        })

    nc = _get_nc()
    res = run_bass_kernel_spmd(nc, in_maps, core_ids=list(range(NC_)),
                               trace=_trace)

    out = np.empty((B, S, D), np.float32)
    for c in range(NC_):
        s0 = (c % 2) * 1024
        out[c // 2, s0:s0 + 1024, :] = res.results[c]["out_shard"]
    if _trace:
        return out, res
    return out
